# revision 13
# baseline (speedup 1.0000x reference)
"""Trainium2 Bass kernel for nn_Mismatch_loss (weighted per-channel MSE loss).

Contract: kernel(**inputs) takes FULL fp32 inputs (net_out, target,
max_positiones of shape [8, 16, 384, 384]) and returns the FULL scalar
output, distributing work across 8 NeuronCores internally (data-parallel
over batch: core b processes image b).

Math per (b, c) channel (spatial reductions over 384*384 = HW elements):
    d   = t - n
    S1  = sum(t)
    S2  = sum(d^2)
    S3  = sum(d^2 * t)
    loss = ALPHA*S3/(S1+eps) + (1-ALPHA)*(S2-S3)/(HW-S1+eps)
Final [B, C] -> scalar runs on host from the gathered per-channel sums.

v2 design (from perfetto analysis of the v1 kernel):
  - The kernel is input-stream-bound.  v1 shipped both tensors as fp16
    over the single HWDGE ring: 9.44 MB at the ~358 GB/s HBM-per-core
    limit = 26.4 us of streaming.  v2 ships `target` as fp8(e4m3) in
    HBM and upcasts to bf16 *inside the DMA* via the gpsimd SWDGE ring
    (dtype-casting DMA), while `net_out` ships as bf16 over the sync
    HWDGE ring.  The two rings drain concurrently through the shared
    SDMA engines, so the bottleneck moves to the SBUF-write fabric
    (~435 GB/s): ~22 us of streaming and much less HBM pressure.
  - All SBUF compute is bf16 (DVE tensor_tensor runs 2x; PE runs
    full-rate bf16; fp8 operands would drop DVE to 1x).
  - Engine balance: DVE does subs + muls (+ two squares), ACT does the
    other squares with fused per-partition accumulation (S2 columns),
    PE does one-hot column-sum matmuls: S1 (psum1), S3 (psum3), and S2
    for the DVE-squared channels (psum2).
  - Channel 15 ships split in two half-F tiles; its halves' matmuls
    accumulate into the same PSUM rows, keeping the post-stream chase
    short.  PSUM reduces run on ACT as soon as their groups close.
"""

import os
import sys

import numpy as np
import ml_dtypes

for _p in ("/opt/trn_rl_repo", "/root/.axon_site/_ro/trn_rl_repo"):
    if os.path.isdir(_p) and _p not in sys.path:
        sys.path.append(_p)

B, C, H, W = 8, 16, 384, 384
HWE = H * W          # 147456 spatial elements per channel
P = 128              # SBUF partitions
F = HWE // P         # 1152 elements per partition per channel
SMOOTH = 1e-6
ALPHA = 0.05

RING = 4
HALF = F // 2        # 576

# slots whose square runs on DVE (d*d tensor_tensor); their S2 goes
# through PE one-hot passes into psum2 rows 0..len-1
DVE_SQ = (5, 8)

# output column layout in out_all [P, OUT_W] fp32
S2COL_15A, S2COL_15B = 15, 16
S2PSUM_COL = 17      # rows 0:len(DVE_SQ) = psum2 reduce (S2 of DVE_SQ slots)
S1_COL = 18          # rows 0:16 = psum1 reduce (sum t per channel)
S3PSUM_COL = 19      # rows 0:16 = psum3 reduce (sum d2*t per channel)
OUT_W = 20

# DMA channel-groups (applies to both tensors); 15 ships as two half-F
GROUPS = [[0], [1], [2, 3], [4, 5], [6, 7], [8, 9], [10, 11], [12, 13], [14]]

_CACHE = {}


def _build_v2(dma_mode):
    import concourse.bass as bass
    import concourse.mybir as mybir

    bf = mybir.dt.bfloat16
    f8 = mybir.dt.float8e4
    f32 = mybir.dt.float32
    Alu = mybir.AluOpType
    Act = mybir.ActivationFunctionType

    nc = bass.Bass("TRN2", target_bir_lowering=False, debug=False, num_devices=1)
    t_dt = f8 if dma_mode == "hybrid" else bf
    t_in = nc.dram_tensor("t_in", [C, P, F], t_dt, kind="ExternalInput")
    n_in = nc.dram_tensor("n_in", [C, P, F], bf, kind="ExternalInput")
    oneh_in = nc.dram_tensor("oneh", [P, 16, 16], bf, kind="ExternalInput")
    out_all = nc.dram_tensor("out_all", [P, OUT_W], f32, kind="ExternalOutput")

    grp_of = {}
    for g, chans in enumerate(GROUPS):
        for c in chans:
            grp_of[c] = g
    NG = len(GROUPS)

    from contextlib import ExitStack

    with ExitStack() as ctx:
        ctx.enter_context(nc.cleanup_on_exit())
        sb = lambda name, shape, dtype: ctx.enter_context(  # noqa: E731
            nc.sbuf_tensor(name, shape, dtype)
        )
        t_sb = {g: sb(f"t_sb{g}", [P, len(ch), F], bf) for g, ch in enumerate(GROUPS)}
        n_sb = {g: sb(f"n_sb{g}", [P, len(ch), F], bf) for g, ch in enumerate(GROUPS)}
        t15 = sb("t15_sb", [P, F], bf)
        n15 = sb("n15_sb", [P, F], bf)
        d_sb = [sb(f"d_sb{k}", [P, F], bf) for k in range(RING)]
        d2_sb = [sb(f"d2_sb{k}", [P, F], bf) for k in range(RING)]
        p_sb = [sb(f"p_sb{k}", [P, F], bf) for k in range(RING)]
        d15 = sb("d15_sb", [P, F], bf)
        d215 = sb("d215_sb", [P, F], bf)
        oneh = sb("oneh_sb", [P, 16, 16], bf)
        outb = sb("outb_sb", [P, OUT_W], f32)
        scratch = sb("scratch_sb", [P, 1], bf)
        red_scr = sb("red_scr_sb", [16, 512], f32)
        psum1 = ctx.enter_context(nc.psum_tensor("psum1", [16, 512], f32))
        psum3 = ctx.enter_context(nc.psum_tensor("psum3", [16, 512], f32))
        psum2 = ctx.enter_context(nc.psum_tensor("psum2", [16, 512], f32))

        sem = nc.alloc_semaphore
        s_t = [sem(f"s_t{g}") for g in range(NG)]
        s_n = [sem(f"s_n{g}") for g in range(NG)]
        s_t15 = [sem("s_t15a"), sem("s_t15b")]
        s_n15 = [sem("s_n15a"), sem("s_n15b")]
        s_oneh = sem("s_oneh")
        s_d = sem("s_d")       # DVE subs (incl. 15a/15b)
        s_sqa = sem("s_sqa")   # ACT squares done (ACT queue order)
        s_sqd = sem("s_sqd")   # DVE squares done (DVE_SQ order)
        s_p = sem("s_p")       # DVE muls done (all slots order)
        s_pet = sem("s_pet")   # PE t-pass slots completed
        s_pep = sem("s_pep")   # PE p-pass slots completed
        s_ped2 = sem("s_ped2")  # PE d2-pass (DVE_SQ) completed
        s_red = sem("s_red")   # psum reduces completed
        s_out = sem("s_out")

        all_slots = list(range(15)) + ["15a", "15b"]
        act_sq_order = [s for s in all_slots if s not in DVE_SQ]
        mul_order = list(all_slots)
        pe_t_order = list(all_slots)
        pe_p_order = list(all_slots)

        sub_pos = {s: i for i, s in enumerate(all_slots)}
        sqa_pos = {s: i for i, s in enumerate(act_sq_order)}
        sqd_pos = {s: i for i, s in enumerate(DVE_SQ)}
        mul_pos = {s: i for i, s in enumerate(mul_order)}
        pep_pos = {s: i for i, s in enumerate(pe_p_order)}

        def sq_done_wait(engine, slot):
            if slot in DVE_SQ:
                engine.wait_ge(s_sqd, sqd_pos[slot] + 1)
            else:
                engine.wait_ge(s_sqa, sqa_pos[slot] + 1)

        def d2_consumed_wait(engine, slot):
            """d2 ring WAR: wait until slot's d2 consumers are done."""
            engine.wait_ge(s_p, mul_pos[slot] + 1)
            if slot in DVE_SQ:
                engine.wait_ge(s_ped2, sqd_pos[slot] + 1)

        def t_ap(s):
            if s == "15a":
                return t15[:, 0:HALF]
            if s == "15b":
                return t15[:, HALF:F]
            g = grp_of[s]
            return t_sb[g][:, GROUPS[g].index(s), :]

        def n_ap(s):
            if s == "15a":
                return n15[:, 0:HALF]
            if s == "15b":
                return n15[:, HALF:F]
            g = grp_of[s]
            return n_sb[g][:, GROUPS[g].index(s), :]

        def d_ap(s):
            if s == "15a":
                return d15[:, 0:HALF]
            if s == "15b":
                return d15[:, HALF:F]
            return d_sb[s % RING][:, :]

        def d2_ap(s):
            if s == "15a":
                return d215[:, 0:HALF]
            if s == "15b":
                return d215[:, HALF:F]
            return d2_sb[s % RING][:, :]

        # ---- input DMAs ----
        t_eng = nc.gpsimd if dma_mode == "hybrid" else nc.sync
        for g, chans in enumerate(GROUPS):
            c0 = chans[0]
            t_eng.dma_start(
                t_sb[g][:, :, :],
                t_in.ap()[c0 : c0 + len(chans)].rearrange("c p f -> p c f"),
            ).then_inc(s_t[g], 16)
        t_eng.dma_start(t15[:, 0:HALF], t_in.ap()[15, :, 0:HALF]).then_inc(
            s_t15[0], 16
        )
        t_eng.dma_start(t15[:, HALF:F], t_in.ap()[15, :, HALF:F]).then_inc(
            s_t15[1], 16
        )

        nc.sync.dma_start(oneh[:, :, :], oneh_in.ap()).then_inc(s_oneh, 16)
        for g, chans in enumerate(GROUPS):
            c0 = chans[0]
            nc.sync.dma_start(
                n_sb[g][:, :, :],
                n_in.ap()[c0 : c0 + len(chans)].rearrange("c p f -> p c f"),
            ).then_inc(s_n[g], 16)
        nc.sync.dma_start(n15[:, 0:HALF], n_in.ap()[15, :, 0:HALF]).then_inc(
            s_n15[0], 16
        )
        nc.sync.dma_start(n15[:, HALF:F], n_in.ap()[15, :, HALF:F]).then_inc(
            s_n15[1], 16
        )

        # ---- DVE: subs + muls (+ DVE_SQ squares), interleaved ----
        def emit_sub(s):
            if s == "15a":
                nc.vector.wait_ge(s_t15[0], 16)
                nc.vector.wait_ge(s_n15[0], 16)
            elif s == "15b":
                nc.vector.wait_ge(s_t15[1], 16)
                nc.vector.wait_ge(s_n15[1], 16)
            else:
                g = grp_of[s]
                if s == GROUPS[g][0]:
                    nc.vector.wait_ge(s_t[g], 16)
                    nc.vector.wait_ge(s_n[g], 16)
                if s >= RING:
                    sq_done_wait(nc.vector, s - RING)  # d ring WAR
            nc.vector.tensor_tensor(
                d_ap(s), t_ap(s), n_ap(s), Alu.subtract
            ).then_inc(s_d, 1)

        def emit_dve_sq(s):
            # d ready (same queue as sub); d2 ring WAR
            if s >= RING:
                d2_consumed_wait(nc.vector, s - RING)
            nc.vector.tensor_tensor(
                d2_ap(s), d_ap(s), d_ap(s), Alu.mult
            ).then_inc(s_sqd, 1)

        def p_ap(s):
            buf = p_sb[pep_pos[s] % RING]
            if s == "15a":
                return buf[:, 0:HALF]
            if s == "15b":
                return buf[:, HALF:F]
            return buf[:, :]

        def emit_mul(s):
            if s not in DVE_SQ:
                sq_done_wait(nc.vector, s)
            if pep_pos[s] >= RING:
                nc.vector.wait_ge(s_pep, pep_pos[s] - RING + 1)  # p ring WAR
            nc.vector.tensor_tensor(
                p_ap(s), d2_ap(s), t_ap(s), Alu.mult
            ).then_inc(s_p, 1)

        SKEW = 2
        mul_q = list(mul_order)
        mi = 0
        for i, s in enumerate(all_slots):
            emit_sub(s)
            if s in DVE_SQ:
                emit_dve_sq(s)
            while mi < len(mul_q) and sub_pos[mul_q[mi]] <= i - SKEW:
                emit_mul(mul_q[mi])
                mi += 1
        while mi < len(mul_q):
            emit_mul(mul_q[mi])
            mi += 1

        # ---- ACT: table preload, squares with accumulation, psum reduces --
        nc.scalar.activation(scratch[:, :], scratch[:, :], Act.Square)
        psum2_red_after = 12  # emit psum2 reduce after this slot's square
        for s in act_sq_order:
            nc.scalar.wait_ge(s_d, sub_pos[s] + 1)
            if isinstance(s, int) and s >= RING:
                d2_consumed_wait(nc.scalar, s - RING)  # d2 ring WAR
            col = s if isinstance(s, int) else (S2COL_15A if s == "15a" else S2COL_15B)
            nc.scalar.activation(
                d2_ap(s),
                d_ap(s),
                Act.Square,
                accum_out=outb[:, col : col + 1],
            ).then_inc(s_sqa, 1)
            if s == psum2_red_after and DVE_SQ:
                nc.scalar.wait_ge(s_ped2, len(DVE_SQ))
                nc.scalar.activation(
                    red_scr[0 : len(DVE_SQ), :],
                    psum2[0 : len(DVE_SQ), :],
                    Act.Copy,
                    accum_out=outb[0 : len(DVE_SQ), S2PSUM_COL : S2PSUM_COL + 1],
                ).then_inc(s_red, 1)
        nc.scalar.wait_ge(s_pet, len(pe_t_order))
        nc.scalar.activation(
            red_scr[:, :], psum1[:, :], Act.Copy,
            accum_out=outb[0:16, S1_COL : S1_COL + 1],
        ).then_inc(s_red, 1)
        nc.scalar.wait_ge(s_pep, len(pe_p_order))
        nc.scalar.activation(
            red_scr[:, :], psum3[:, :], Act.Copy,
            accum_out=outb[0:16, S3PSUM_COL : S3PSUM_COL + 1],
        ).then_inc(s_red, 1)

        # ---- PE: one-hot column-sum matmuls ----
        CHUNKS = (512, 512, 128)
        HCHUNKS = (512, 64)

        def chunks_of(s):
            return HCHUNKS if s in ("15a", "15b") else CHUNKS

        def w_of(s, k=None):
            c = 15 if s in ("15a", "15b") else s
            if k is not None:
                c = k
            return oneh[:, c, :]

        def emit_pe_pass(s, psum, src_ap, first, last, sem_, w_idx=None):
            off = 0
            chs = chunks_of(s)
            total = sum(chs)
            for wdt in chs:
                mm = nc.tensor.matmul(
                    psum[:, 0:wdt],
                    lhsT=w_of(s, w_idx),
                    rhs=src_ap[:, off : off + wdt],
                    start=(first and off == 0),
                    stop=(last and off + wdt == total),
                    skip_group_check=True,
                )
                off += wdt
            mm.then_inc(sem_, 1)

        def emit_pe_t(s):
            if s == "15a":
                nc.tensor.wait_ge(s_t15[0], 16)
            elif s == "15b":
                nc.tensor.wait_ge(s_t15[1], 16)
            else:
                nc.tensor.wait_ge(s_t[grp_of[s]], 16)
            emit_pe_pass(
                s, psum1, t_ap(s),
                first=(s == pe_t_order[0]), last=(s == pe_t_order[-1]),
                sem_=s_pet,
            )

        def emit_pe_p(s):
            nc.tensor.wait_ge(s_p, mul_pos[s] + 1)
            emit_pe_pass(
                s, psum3, p_ap(s),
                first=(s == pe_p_order[0]), last=(s == pe_p_order[-1]),
                sem_=s_pep,
            )

        def emit_pe_d2(s):
            nc.tensor.wait_ge(s_sqd, sqd_pos[s] + 1)
            emit_pe_pass(
                s, psum2, d2_ap(s),
                first=(sqd_pos[s] == 0), last=(sqd_pos[s] == len(DVE_SQ) - 1),
                sem_=s_ped2, w_idx=sqd_pos[s],
            )

        nc.tensor.wait_ge(s_oneh, 16)
        p_q = list(pe_p_order)
        pi = 0
        for i, s in enumerate(pe_t_order):
            emit_pe_t(s)
            if s in DVE_SQ:
                emit_pe_d2(s)
            while pi < len(p_q) and sub_pos[p_q[pi]] <= i - 1:
                emit_pe_p(p_q[pi])
                pi += 1
        while pi < len(p_q):
            emit_pe_p(p_q[pi])
            pi += 1

        # ---- SP: final output DMA after all stats complete ----
        nc.sync.wait_ge(s_sqa, len(act_sq_order))
        nc.sync.wait_ge(s_red, 3 if DVE_SQ else 2)
        nc.sync.dma_start(out_all.ap(), outb[:, :]).then_inc(s_out, 16)
        nc.sync.wait_ge(s_out, 16)

    return nc


def _get_nc():
    mode = os.environ.get("BASS_V2_DMA", "hybrid")
    key = f"v2_{mode}"
    if key not in _CACHE:
        _CACHE[key] = _build_v2(mode)
    return _CACHE[key]


def _make_oneh():
    oneh = np.zeros((P, 16, 16), dtype=ml_dtypes.bfloat16)
    for c in range(C):
        oneh[:, c, c] = 1.0
    return oneh


def kernel(net_out, target, max_positiones):
    from concourse import bass_utils

    mode = os.environ.get("BASS_V2_DMA", "hybrid")
    nc = _get_nc()

    t32 = np.ascontiguousarray(np.asarray(target, np.float32).reshape(B, C, P, F))
    n32 = np.ascontiguousarray(np.asarray(net_out, np.float32).reshape(B, C, P, F))
    if mode == "hybrid":
        t_h = t32.astype(ml_dtypes.float8_e4m3)
    else:
        t_h = t32.astype(ml_dtypes.bfloat16)
    n_h = n32.astype(ml_dtypes.bfloat16)
    oneh = _make_oneh()

    in_maps = [
        {"t_in": t_h[b], "n_in": n_h[b], "oneh": oneh} for b in range(B)
    ]

    last_err = None
    for _attempt in range(4):
        try:
            res = bass_utils.run_bass_kernel_spmd(
                nc, in_maps, core_ids=list(range(8))
            )
            break
        except Exception as e:  # noqa: BLE001
            last_err = e
            import time as _time

            _time.sleep(3.0)
            try:
                import jax

                jax.clear_caches()
                jax.extend.backend.clear_backends()
            except Exception:  # noqa: BLE001
                pass
            _time.sleep(2.0)
    else:
        raise last_err

    S1 = np.empty((B, C), np.float64)
    S2 = np.empty((B, C), np.float64)
    S3 = np.empty((B, C), np.float64)
    for b in range(B):
        out = np.asarray(res.results[b]["out_all"], dtype=np.float64)
        S1[b] = out[:16, S1_COL]
        S3[b] = out[:16, S3PSUM_COL]
        for s in range(15):
            if s in DVE_SQ:
                S2[b, s] = out[DVE_SQ.index(s), S2PSUM_COL]
            else:
                S2[b, s] = out[:, s].sum()
        S2[b, 15] = out[:, S2COL_15A].sum() + out[:, S2COL_15B].sum()

    m1, m2, d1 = S3, S2 - S3, S1
    d2n = float(HWE) - d1
    loss = ALPHA * m1 / (d1 + SMOOTH) + (1.0 - ALPHA) * m2 / (d2n + SMOOTH)

    # active-mask: S1 != 0 implies max(target[b,c]) != 0 for non-negative
    # targets; the S1 == 0 corner is resolved exactly on host.
    active = S1 != 0.0
    for b, c in zip(*np.nonzero(~active)):
        mt = np.max(target[b, c])
        mmp = np.max(max_positiones[b, c])
        active[b, c] = not (mt == 0.0 and mmp == 0.0)

    losses = np.where(active, loss, 0.0)
    count = (losses != 0.0).sum(axis=1).astype(np.float64)
    img_losses = losses.sum(axis=1) / count
    return np.float32(img_losses.mean())


# revision 14
# speedup vs baseline: 2.8087x; 2.8087x over previous
"""Trainium2 Bass kernel for nn_Mismatch_loss (weighted per-channel MSE loss).

Contract: kernel(**inputs) takes FULL fp32 inputs (net_out, target,
max_positiones of shape [8, 16, 384, 384]) and returns the FULL scalar
output, distributing work across 8 NeuronCores internally (data-parallel
over batch: core b processes image b).

Math per (b, c) channel (spatial reductions over 384*384 = HW elements):
    d   = t - n
    S1  = sum(t)
    S2  = sum(d^2)
    S3  = sum(d^2 * t)
    loss = ALPHA*S3/(S1+eps) + (1-ALPHA)*(S2-S3)/(HW-S1+eps)
Final [B, C] -> scalar runs on host from the gathered per-channel sums.

v2 design (from perfetto analysis of the v1 kernel):
  - The kernel is input-stream-bound.  v1 shipped both tensors as fp16
    over the single HWDGE ring: 9.44 MB at the ~358 GB/s HBM-per-core
    limit = 26.4 us of streaming.  v2 ships `target` as fp8(e4m3) in
    HBM and upcasts to bf16 *inside the DMA* via the gpsimd SWDGE ring
    (dtype-casting DMA), while `net_out` ships as bf16 over the sync
    HWDGE ring.  The two rings drain concurrently through the shared
    SDMA engines, so the bottleneck moves to the SBUF-write fabric
    (~435 GB/s): ~22 us of streaming and much less HBM pressure.
  - All SBUF compute is bf16 (DVE tensor_tensor runs 2x; PE runs
    full-rate bf16; fp8 operands would drop DVE to 1x).
  - Engine balance: DVE does subs + muls (+ two squares), ACT does the
    other squares with fused per-partition accumulation (S2 columns),
    PE does one-hot column-sum matmuls: S1 (psum1), S3 (psum3), and S2
    for the DVE-squared channels (psum2).
  - Channel 15 ships split in two half-F tiles; its halves' matmuls
    accumulate into the same PSUM rows, keeping the post-stream chase
    short.  PSUM reduces run on ACT as soon as their groups close.
"""

import os
import sys

import numpy as np
import ml_dtypes

for _p in ("/opt/trn_rl_repo", "/root/.axon_site/_ro/trn_rl_repo"):
    if os.path.isdir(_p) and _p not in sys.path:
        sys.path.append(_p)

B, C, H, W = 8, 16, 384, 384
HWE = H * W          # 147456 spatial elements per channel
P = 128              # SBUF partitions
F = HWE // P         # 1152 elements per partition per channel
SMOOTH = 1e-6
ALPHA = 0.05

RING = 4
HALF = F // 2        # 576

# slots whose square runs on DVE (d*d tensor_tensor); their S2 goes
# through PE one-hot passes into psum2 rows 0..len-1
DVE_SQ = (5, 8)

# output column layout in out_all [P, OUT_W] fp32
S2COL_15A, S2COL_15B = 15, 16
S2PSUM_COL = 17      # rows 0:len(DVE_SQ) = psum2 reduce (S2 of DVE_SQ slots)
S1_COL = 18          # rows 0:16 = psum1 reduce (sum t per channel)
S3PSUM_COL = 19      # rows 0:16 = psum3 reduce (sum d2*t per channel)
OUT_W = 20

# DMA channel-groups (applies to both tensors); 15 ships as two half-F
GROUPS = [[0], [1], [2, 3], [4, 5], [6, 7], [8, 9], [10, 11], [12, 13], [14]]

_CACHE = {}


def _build_v2(dma_mode):
    import concourse.bass as bass
    import concourse.mybir as mybir

    bf = mybir.dt.bfloat16
    f8 = mybir.dt.float8e4
    f32 = mybir.dt.float32
    Alu = mybir.AluOpType
    Act = mybir.ActivationFunctionType

    nc = bass.Bass("TRN2", target_bir_lowering=False, debug=False, num_devices=1)
    t_dt = f8 if dma_mode == "hybrid" else bf
    t_in = nc.dram_tensor("t_in", [C, P, F], t_dt, kind="ExternalInput")
    n_in = nc.dram_tensor("n_in", [C, P, F], bf, kind="ExternalInput")
    oneh_in = nc.dram_tensor("oneh", [P, 16, 16], bf, kind="ExternalInput")
    out_all = nc.dram_tensor("out_all", [P, OUT_W], f32, kind="ExternalOutput")

    grp_of = {}
    for g, chans in enumerate(GROUPS):
        for c in chans:
            grp_of[c] = g
    NG = len(GROUPS)

    from contextlib import ExitStack

    with ExitStack() as ctx:
        ctx.enter_context(nc.cleanup_on_exit())
        sb = lambda name, shape, dtype: ctx.enter_context(  # noqa: E731
            nc.sbuf_tensor(name, shape, dtype)
        )
        t_sb = {g: sb(f"t_sb{g}", [P, len(ch), F], bf) for g, ch in enumerate(GROUPS)}
        n_sb = {g: sb(f"n_sb{g}", [P, len(ch), F], bf) for g, ch in enumerate(GROUPS)}
        t15 = sb("t15_sb", [P, F], bf)
        n15 = sb("n15_sb", [P, F], bf)
        d_sb = [sb(f"d_sb{k}", [P, F], bf) for k in range(RING)]
        d2_sb = [sb(f"d2_sb{k}", [P, F], bf) for k in range(RING)]
        p_sb = [sb(f"p_sb{k}", [P, F], bf) for k in range(RING)]
        d15 = sb("d15_sb", [P, F], bf)
        d215 = sb("d215_sb", [P, F], bf)
        oneh = sb("oneh_sb", [P, 16, 16], bf)
        outb = sb("outb_sb", [P, OUT_W], f32)
        scratch = sb("scratch_sb", [P, 1], bf)
        red_scr = sb("red_scr_sb", [16, 512], f32)
        psum1 = ctx.enter_context(nc.psum_tensor("psum1", [16, 512], f32))
        psum3 = ctx.enter_context(nc.psum_tensor("psum3", [16, 512], f32))
        psum2 = ctx.enter_context(nc.psum_tensor("psum2", [16, 512], f32))

        sem = nc.alloc_semaphore
        s_t = [sem(f"s_t{g}") for g in range(NG)]
        s_n = [sem(f"s_n{g}") for g in range(NG)]
        s_t15 = [sem("s_t15a"), sem("s_t15b")]
        s_n15 = [sem("s_n15a"), sem("s_n15b")]
        s_oneh = sem("s_oneh")
        s_d = sem("s_d")       # DVE subs (incl. 15a/15b)
        s_sqa = sem("s_sqa")   # ACT squares done (ACT queue order)
        s_sqd = sem("s_sqd")   # DVE squares done (DVE_SQ order)
        s_p = sem("s_p")       # DVE muls done (all slots order)
        s_pet = sem("s_pet")   # PE t-pass slots completed
        s_pep = sem("s_pep")   # PE p-pass slots completed
        s_ped2 = sem("s_ped2")  # PE d2-pass (DVE_SQ) completed
        s_red = sem("s_red")   # psum reduces completed
        s_out = sem("s_out")

        all_slots = list(range(15)) + ["15a", "15b"]
        act_sq_order = [s for s in all_slots if s not in DVE_SQ]
        mul_order = list(all_slots)
        pe_t_order = list(all_slots)
        pe_p_order = list(all_slots)

        sub_pos = {s: i for i, s in enumerate(all_slots)}
        sqa_pos = {s: i for i, s in enumerate(act_sq_order)}
        sqd_pos = {s: i for i, s in enumerate(DVE_SQ)}
        mul_pos = {s: i for i, s in enumerate(mul_order)}
        pep_pos = {s: i for i, s in enumerate(pe_p_order)}

        def sq_done_wait(engine, slot):
            if slot in DVE_SQ:
                engine.wait_ge(s_sqd, sqd_pos[slot] + 1)
            else:
                engine.wait_ge(s_sqa, sqa_pos[slot] + 1)

        def d2_consumed_wait(engine, slot):
            """d2 ring WAR: wait until slot's d2 consumers are done."""
            engine.wait_ge(s_p, mul_pos[slot] + 1)
            if slot in DVE_SQ:
                engine.wait_ge(s_ped2, sqd_pos[slot] + 1)

        def t_ap(s):
            if s == "15a":
                return t15[:, 0:HALF]
            if s == "15b":
                return t15[:, HALF:F]
            g = grp_of[s]
            return t_sb[g][:, GROUPS[g].index(s), :]

        def n_ap(s):
            if s == "15a":
                return n15[:, 0:HALF]
            if s == "15b":
                return n15[:, HALF:F]
            g = grp_of[s]
            return n_sb[g][:, GROUPS[g].index(s), :]

        def d_ap(s):
            if s == "15a":
                return d15[:, 0:HALF]
            if s == "15b":
                return d15[:, HALF:F]
            return d_sb[s % RING][:, :]

        def d2_ap(s):
            if s == "15a":
                return d215[:, 0:HALF]
            if s == "15b":
                return d215[:, HALF:F]
            return d2_sb[s % RING][:, :]

        # ---- input DMAs ----
        # hybrid: t on the gpsimd SWDGE ring (cast fp8->bf16), n on the
        # sync HWDGE ring -- the rings drain concurrently.  bf16: both on
        # sync, t/n interleaved per group so channel g's pair lands early.
        t_eng = nc.gpsimd if dma_mode == "hybrid" else nc.sync

        def t_dma(g):
            c0 = GROUPS[g][0]
            t_eng.dma_start(
                t_sb[g][:, :, :],
                t_in.ap()[c0 : c0 + len(GROUPS[g])].rearrange("c p f -> p c f"),
            ).then_inc(s_t[g], 16)

        def n_dma(g):
            c0 = GROUPS[g][0]
            nc.sync.dma_start(
                n_sb[g][:, :, :],
                n_in.ap()[c0 : c0 + len(GROUPS[g])].rearrange("c p f -> p c f"),
            ).then_inc(s_n[g], 16)

        nc.sync.dma_start(oneh[:, :, :], oneh_in.ap()).then_inc(s_oneh, 16)
        for g in range(NG):
            t_dma(g)
            n_dma(g)
        t_eng.dma_start(t15[:, 0:HALF], t_in.ap()[15, :, 0:HALF]).then_inc(
            s_t15[0], 16
        )
        nc.sync.dma_start(n15[:, 0:HALF], n_in.ap()[15, :, 0:HALF]).then_inc(
            s_n15[0], 16
        )
        t_eng.dma_start(t15[:, HALF:F], t_in.ap()[15, :, HALF:F]).then_inc(
            s_t15[1], 16
        )
        nc.sync.dma_start(n15[:, HALF:F], n_in.ap()[15, :, HALF:F]).then_inc(
            s_n15[1], 16
        )

        # ---- DVE: subs + muls (+ DVE_SQ squares), interleaved ----
        def emit_sub(s):
            if s == "15a":
                nc.vector.wait_ge(s_t15[0], 16)
                nc.vector.wait_ge(s_n15[0], 16)
            elif s == "15b":
                nc.vector.wait_ge(s_t15[1], 16)
                nc.vector.wait_ge(s_n15[1], 16)
            else:
                g = grp_of[s]
                if s == GROUPS[g][0]:
                    nc.vector.wait_ge(s_t[g], 16)
                    nc.vector.wait_ge(s_n[g], 16)
                if s >= RING:
                    sq_done_wait(nc.vector, s - RING)  # d ring WAR
            nc.vector.tensor_tensor(
                d_ap(s), t_ap(s), n_ap(s), Alu.subtract
            ).then_inc(s_d, 1)

        def emit_dve_sq(s):
            # d ready (same queue as sub); d2 ring WAR
            if s >= RING:
                d2_consumed_wait(nc.vector, s - RING)
            nc.vector.tensor_tensor(
                d2_ap(s), d_ap(s), d_ap(s), Alu.mult
            ).then_inc(s_sqd, 1)

        def p_ap(s):
            buf = p_sb[pep_pos[s] % RING]
            if s == "15a":
                return buf[:, 0:HALF]
            if s == "15b":
                return buf[:, HALF:F]
            return buf[:, :]

        def emit_mul(s):
            if s not in DVE_SQ:
                sq_done_wait(nc.vector, s)
            if pep_pos[s] >= RING:
                nc.vector.wait_ge(s_pep, pep_pos[s] - RING + 1)  # p ring WAR
            nc.vector.tensor_tensor(
                p_ap(s), d2_ap(s), t_ap(s), Alu.mult
            ).then_inc(s_p, 1)

        SKEW = 2
        mul_q = list(mul_order)
        mi = 0
        for i, s in enumerate(all_slots):
            emit_sub(s)
            if s in DVE_SQ:
                emit_dve_sq(s)
            while mi < len(mul_q) and sub_pos[mul_q[mi]] <= i - SKEW:
                emit_mul(mul_q[mi])
                mi += 1
        while mi < len(mul_q):
            emit_mul(mul_q[mi])
            mi += 1

        # ---- ACT: table preload, squares with accumulation, psum reduces --
        nc.scalar.activation(scratch[:, :], scratch[:, :], Act.Square)
        psum2_red_after = 12  # emit psum2 reduce after this slot's square
        for s in act_sq_order:
            nc.scalar.wait_ge(s_d, sub_pos[s] + 1)
            if isinstance(s, int) and s >= RING:
                d2_consumed_wait(nc.scalar, s - RING)  # d2 ring WAR
            col = s if isinstance(s, int) else (S2COL_15A if s == "15a" else S2COL_15B)
            nc.scalar.activation(
                d2_ap(s),
                d_ap(s),
                Act.Square,
                accum_out=outb[:, col : col + 1],
            ).then_inc(s_sqa, 1)
            if s == psum2_red_after and DVE_SQ:
                nc.scalar.wait_ge(s_ped2, len(DVE_SQ))
                nc.scalar.activation(
                    red_scr[0 : len(DVE_SQ), :],
                    psum2[0 : len(DVE_SQ), :],
                    Act.Copy,
                    accum_out=outb[0 : len(DVE_SQ), S2PSUM_COL : S2PSUM_COL + 1],
                ).then_inc(s_red, 1)
        nc.scalar.wait_ge(s_pet, len(pe_t_order))
        nc.scalar.activation(
            red_scr[:, :], psum1[:, :], Act.Copy,
            accum_out=outb[0:16, S1_COL : S1_COL + 1],
        ).then_inc(s_red, 1)
        nc.scalar.wait_ge(s_pep, len(pe_p_order))
        nc.scalar.activation(
            red_scr[:, :], psum3[:, :], Act.Copy,
            accum_out=outb[0:16, S3PSUM_COL : S3PSUM_COL + 1],
        ).then_inc(s_red, 1)

        # ---- PE: one-hot column-sum matmuls ----
        CHUNKS = (512, 512, 128)
        HCHUNKS = (512, 64)

        def chunks_of(s):
            return HCHUNKS if s in ("15a", "15b") else CHUNKS

        def w_of(s, k=None):
            c = 15 if s in ("15a", "15b") else s
            if k is not None:
                c = k
            return oneh[:, c, :]

        def emit_pe_pass(s, psum, src_ap, first, last, sem_, w_idx=None):
            off = 0
            chs = chunks_of(s)
            total = sum(chs)
            for wdt in chs:
                mm = nc.tensor.matmul(
                    psum[:, 0:wdt],
                    lhsT=w_of(s, w_idx),
                    rhs=src_ap[:, off : off + wdt],
                    start=(first and off == 0),
                    stop=(last and off + wdt == total),
                    skip_group_check=True,
                )
                off += wdt
            mm.then_inc(sem_, 1)

        def emit_pe_t(s):
            if s == "15a":
                nc.tensor.wait_ge(s_t15[0], 16)
            elif s == "15b":
                nc.tensor.wait_ge(s_t15[1], 16)
            else:
                nc.tensor.wait_ge(s_t[grp_of[s]], 16)
            emit_pe_pass(
                s, psum1, t_ap(s),
                first=(s == pe_t_order[0]), last=(s == pe_t_order[-1]),
                sem_=s_pet,
            )

        def emit_pe_p(s):
            nc.tensor.wait_ge(s_p, mul_pos[s] + 1)
            emit_pe_pass(
                s, psum3, p_ap(s),
                first=(s == pe_p_order[0]), last=(s == pe_p_order[-1]),
                sem_=s_pep,
            )

        def emit_pe_d2(s):
            nc.tensor.wait_ge(s_sqd, sqd_pos[s] + 1)
            emit_pe_pass(
                s, psum2, d2_ap(s),
                first=(sqd_pos[s] == 0), last=(sqd_pos[s] == len(DVE_SQ) - 1),
                sem_=s_ped2, w_idx=sqd_pos[s],
            )

        nc.tensor.wait_ge(s_oneh, 16)
        p_q = list(pe_p_order)
        pi = 0
        for i, s in enumerate(pe_t_order):
            emit_pe_t(s)
            if s in DVE_SQ:
                emit_pe_d2(s)
            while pi < len(p_q) and sub_pos[p_q[pi]] <= i - 1:
                emit_pe_p(p_q[pi])
                pi += 1
        while pi < len(p_q):
            emit_pe_p(p_q[pi])
            pi += 1

        # ---- SP: final output DMA after all stats complete ----
        nc.sync.wait_ge(s_sqa, len(act_sq_order))
        nc.sync.wait_ge(s_red, 3 if DVE_SQ else 2)
        nc.sync.dma_start(out_all.ap(), outb[:, :]).then_inc(s_out, 16)
        nc.sync.wait_ge(s_out, 16)

    return nc


def _get_nc():
    mode = os.environ.get("BASS_V2_DMA", "hybrid")
    key = f"v2_{mode}"
    if key not in _CACHE:
        _CACHE[key] = _build_v2(mode)
    return _CACHE[key]


def _make_oneh():
    oneh = np.zeros((P, 16, 16), dtype=ml_dtypes.bfloat16)
    for c in range(C):
        oneh[:, c, c] = 1.0
    return oneh


def kernel(net_out, target, max_positiones):
    from concourse import bass_utils

    mode = os.environ.get("BASS_V2_DMA", "hybrid")
    nc = _get_nc()

    t32 = np.ascontiguousarray(np.asarray(target, np.float32).reshape(B, C, P, F))
    n32 = np.ascontiguousarray(np.asarray(net_out, np.float32).reshape(B, C, P, F))
    if mode == "hybrid":
        t_h = t32.astype(ml_dtypes.float8_e4m3)
    else:
        t_h = t32.astype(ml_dtypes.bfloat16)
    n_h = n32.astype(ml_dtypes.bfloat16)
    oneh = _make_oneh()

    in_maps = [
        {"t_in": t_h[b], "n_in": n_h[b], "oneh": oneh} for b in range(B)
    ]

    last_err = None
    for _attempt in range(4):
        try:
            res = bass_utils.run_bass_kernel_spmd(
                nc, in_maps, core_ids=list(range(8))
            )
            break
        except Exception as e:  # noqa: BLE001
            last_err = e
            import time as _time

            _time.sleep(3.0)
            try:
                import jax

                jax.clear_caches()
                jax.extend.backend.clear_backends()
            except Exception:  # noqa: BLE001
                pass
            _time.sleep(2.0)
    else:
        raise last_err

    S1 = np.empty((B, C), np.float64)
    S2 = np.empty((B, C), np.float64)
    S3 = np.empty((B, C), np.float64)
    for b in range(B):
        out = np.asarray(res.results[b]["out_all"], dtype=np.float64)
        S1[b] = out[:16, S1_COL]
        S3[b] = out[:16, S3PSUM_COL]
        for s in range(15):
            if s in DVE_SQ:
                S2[b, s] = out[DVE_SQ.index(s), S2PSUM_COL]
            else:
                S2[b, s] = out[:, s].sum()
        S2[b, 15] = out[:, S2COL_15A].sum() + out[:, S2COL_15B].sum()

    m1, m2, d1 = S3, S2 - S3, S1
    d2n = float(HWE) - d1
    loss = ALPHA * m1 / (d1 + SMOOTH) + (1.0 - ALPHA) * m2 / (d2n + SMOOTH)

    # active-mask: S1 != 0 implies max(target[b,c]) != 0 for non-negative
    # targets; the S1 == 0 corner is resolved exactly on host.
    active = S1 != 0.0
    for b, c in zip(*np.nonzero(~active)):
        mt = np.max(target[b, c])
        mmp = np.max(max_positiones[b, c])
        active[b, c] = not (mt == 0.0 and mmp == 0.0)

    losses = np.where(active, loss, 0.0)
    count = (losses != 0.0).sum(axis=1).astype(np.float64)
    img_losses = losses.sum(axis=1) / count
    return np.float32(img_losses.mean())
